# revision 38
# baseline (speedup 1.0000x reference)
"""MultiHeadAttention Trainium2 Bass kernel (8-core SPMD), v4.

Problem: B=2, S=2048, DIM=1024, H=16 heads (dh=64), fp32 reference.
Sharding: core c handles batch b = c//4 and 4 heads ho = 4*(c%4)..+4
(data-parallel over batch x tensor-parallel over heads).

Key HW facts driving the structure (measured on this part):
  - Each dma_start costs ~650ns of serialized trigger time on its issuing
    engine (only sync/scalar/gpsimd can trigger); per-queue DMA bandwidth
    caps at ~23GB/s regardless of packet size; 16 HW queues ~368GB/s
    aggregate. So: weights are PACKED INTO the x tensors host-side (zero
    extra triggers), each x tensor is split into 16 transfer pieces to
    fill all queues, and triggers are spread across sync+scalar+gpsimd.
  - ACT exp is the compute wall: 128 x (352+1024)/1.2 = 147us. Everything
    else (PE ~130us, DVE ~115us, 25MB DMA) must hide under it.
  - One PSUM accumulation group per 2KB bank; 8 banks total. All phases
    share one pool pair (tags sc/po, 4 tiles of [128,1024]). K-projection
    runs c-outer over all 8 banks chasing the xk arrivals; V-projection
    runs as 4 waves of 4 kt on the po tag only, interleaved into qt0's
    first attention iterations, so the scores->exp stream never sits
    behind it in the PE FIFO and the sc rotation stays free for scores.
  - Load order: xk/xv interleaved 2:1 on sync (xk = exp critical path
    finishes first, xv right behind for the V waves), qt0 masks split in
    4 pieces each, xq-n0+wq via gpsimd, xq-rest delayed behind a gpsimd
    stall. exp starts ~51us in; masks prefetch 2 groups ahead in-loop.
  - PV is emitted `lag` kts late (4 for qt0, 1 after) so exp<-scores
    never waits on PV/mask work through the PE FIFO.
  - V bias is folded out: O_norm = PV_raw/sums + bv, so bv only shifts
    the output by bv @ Wo.T, added host-side. Saves 64 DVE bias ops.
  - Per-qt softmax normalization (reciprocal via DMA gather/broadcast)
    overlaps the next qt's attention; all Q-projections for qt>=1 are
    emitted during qt0's ramp where exp is gated by the V waves anyway.
  - Output projection is q-quarter-major so only 1/4 of it waits on the
    serial qt3 1/sums chain; casts split across DVE+ACT, stores across
    sync+scalar queues.

Measured: 268us (baseline 332us/300us), rel err 2.4e-3 vs fp32 ref.
"""

import os
import sys

sys.path.insert(0, "/opt/trn_rl_repo")
os.environ.setdefault("MYCRO_LOCAL_CACHE", "1")

import numpy as np

import concourse.bass as bass
import concourse.bacc as bacc
import concourse.tile as tile
from concourse import mybir
from concourse import bass_utils

F32 = mybir.dt.float32
BF16 = mybir.dt.bfloat16
NP_BF16 = mybir.dt.np(BF16)

B, S, DIM = 2, 2048, 1024
H = 16
DH = 64
SCALE = 1.0 / (DIM ** 0.5)
N_CORES = 8
HPC = 4          # heads per core
QT = S // 512    # 4 q-chunks of 512
KT = S // 128    # 16 k-tiles of 128
CT = DIM // 128  # 8 contraction tiles for projections
KG = 4           # kt-tiles per mask group

XKW = 2048 + 256          # xk chunk | wk chunk
XVW = 2048 + 256 + 256    # xv chunk | wv chunk | wo_flat chunk
XQW = 2048 + 256          # xq chunk | wq chunk

# vh_aug per-kt layout: per pair p (2 local pairs):
#   A block: [vh_A(64) | ones(1)]                 at cols p*193 + [0, 65)
#   B block: [zeros(32) | ones(1) | zeros(31) | vh_B(64)] at cols p*193 + [65, 193)
# wv host columns are permuted (p0A, p1A, p0B, p1B) so the psum->vha
# evacuation is 2 strided copies instead of 4.
VHA_W = 386


def build_nc():
    nc = bacc.Bacc("TRN2", target_bir_lowering=False)

    xq_d = nc.declare_dram_parameter("xq", [CT, 128, XQW], BF16, isOutput=False)
    xk_d = nc.declare_dram_parameter("xk", [CT, 128, XKW], BF16, isOutput=False)
    xv_d = nc.declare_dram_parameter("xv", [CT, 128, XVW], BF16, isOutput=False)
    bqk_d = nc.declare_dram_parameter("bqk", [128, 4], F32, isOutput=False)
    mk_d = nc.declare_dram_parameter("mk", [QT, KG, 128, 2048], BF16, isOutput=False)
    yt_d = nc.declare_dram_parameter("yt", [8, 4, 128, 512], BF16, isOutput=True)
    # per-qt reciprocal staging: rows 0-1 = A-heads (p0,p1), 2-3 = B-heads
    rscr_d = nc.dram_tensor("rscr", [QT, 4, 512], BF16)

    with tile.TileContext(nc) as tc:
        with tc.tile_pool(name="persist", bufs=1) as singles:
            # PE warmup filler source -- first gpsimd op so it's ready fast
            warm = singles.tile([128, 512], BF16, tag="warm", name="warm")
            nc.gpsimd.memset(warm[:, :], 0.0)

            # preload the exp table set on ACT before its DMA triggers
            etw = singles.tile([1, 2], BF16, tag="etw", name="etw")
            nc.scalar.activation(out=etw, in_=warm[0:1, 0:2],
                                 func=mybir.ActivationFunctionType.Exp)

            def alloc_x(tag, width):
                return [singles.tile([128, width], BF16, tag=f"{tag}{c}",
                                     name=f"{tag}{c}") for c in range(CT)]

            # xv on sync, xk on scalar: two parallel trigger streams from t=0.
            # Queue collisions between the streams average out (all of this
            # data is needed before attention starts anyway).
            # xk first (the scores->exp critical path), then qt0's first two
            # masks, then xv (the V-proj path has pt-buffer slack to absorb
            # its later arrival). All on sync's queue round-robin so the
            # per-queue FIFO order matches this priority.
            bqk_sb = singles.tile([128, 4], F32, tag="bqk", name="bqk")
            nc.sync.dma_start(out=bqk_sb, in_=bqk_d[:, :])
            # xk and xv interleaved 2:1 on sync's queue round-robin: xk
            # (scores critical path) finishes ~2/3 through the stream, and
            # xv is close enough behind that the V-proj waves (which need
            # full xv) land just as the attention pipeline first needs vha
            xk_sb = alloc_x("xk", XKW)
            xv_sb = alloc_x("xv", XVW)
            xv_pieces = [(c, 0, 1280) for c in range(CT)] + \
                        [(c, 1280, XVW) for c in range(CT)]
            for c in range(CT):
                nc.sync.dma_start(out=xk_sb[c][:, 0:1152], in_=xk_d[c][:, 0:1152])
                nc.sync.dma_start(out=xk_sb[c][:, 1152:XKW],
                                  in_=xk_d[c][:, 1152:XKW])
                vc, v0, v1 = xv_pieces[c]
                nc.sync.dma_start(out=xv_sb[vc][:, v0:v1], in_=xv_d[vc][:, v0:v1])
            for vc, v0, v1 in xv_pieces[CT:]:
                nc.sync.dma_start(out=xv_sb[vc][:, v0:v1], in_=xv_d[vc][:, v0:v1])

            mask_tiles = {}

            def load_mask(qt, g):
                # 4-piece split: a whole 512KB tile on one ~23GB/s queue
                # takes 22us and always arrives late
                t = singles.tile([128, 2048], BF16, tag="mask", name="mask",
                                 bufs=3)
                for i in range(4):
                    nc.sync.dma_start(
                        out=t[:, i * 512:(i + 1) * 512],
                        in_=mk_d[qt, g][:, i * 512:(i + 1) * 512])
                mask_tiles[(qt, g)] = t

            def want_mask(gi):
                if gi < QT * KG:
                    qt, g = gi // KG, gi % KG
                    if (qt, g) not in mask_tiles:
                        load_mask(qt, g)

            want_mask(0)
            want_mask(1)

            # xq n=0 cols + wq via gpsimd triggers (interleaves with xk/xv)
            xq_sb = alloc_x("xq", XQW)
            for i in range(6):  # ~3us stall: let xk claim queue slots first
                nc.gpsimd.memset(warm[:, :], 0.0)
            for c in range(CT):
                nc.gpsimd.dma_start(out=xq_sb[c][:, 0:512], in_=xq_d[c][:, 0:512])
            for c in range(CT):
                nc.gpsimd.dma_start(out=xq_sb[c][:, 2048:XQW],
                                    in_=xq_d[c][:, 2048:XQW])

            def wk(c, m):
                return xk_sb[c][:, 2048 + m * 128:2048 + (m + 1) * 128]

            def wv(c):
                return xv_sb[c][:, 2048:2304]

            def wo(p, ot):
                col = p * 1024 + ot * 128
                return xv_sb[col // 256][:, 2304 + col % 256:2304 + col % 256 + 128]

            def wq(c, m):
                return xq_sb[c][:, 2048 + m * 128:2048 + (m + 1) * 128]

            # ---- persistent intermediates ----
            qhT = [[singles.tile([128, 512], BF16, tag=f"qhT{m}_{n}",
                                 name=f"qhT{m}_{n}") for n in range(QT)]
                   for m in range(2)]
            khT = [[singles.tile([128, 512], BF16, tag=f"khT{m}_{n}",
                                 name=f"khT{m}_{n}") for n in range(QT)]
                   for m in range(2)]
            OT = [singles.tile([128, S], BF16, tag=f"OT{m}", name=f"OT{m}") for m in range(2)]
            vha = [singles.tile([128, VHA_W], BF16, tag=f"vha{kt}",
                                name=f"vha{kt}") for kt in range(KT)]
            # cols: qt*1024 + p*512 + q  (p-adjacent: flat per-qt gather reads)
            sums_stage = singles.tile([128, 2 * S], BF16, tag="sums_stage",
                                      name="sums_stage")

            for kt in range(KT):
                for p in range(2):
                    base = p * 193
                    nc.gpsimd.memset(vha[kt][:, base + 64:base + 65], 1.0)
                    nc.gpsimd.memset(vha[kt][:, base + 97:base + 98], 1.0)
                    nc.gpsimd.memset(vha[kt][:, base + 65:base + 97], 0.0)
                    nc.gpsimd.memset(vha[kt][:, base + 98:base + 129], 0.0)

            # stall gpsimd ~12us so the xq n=1..3 triggers fire after the
            # critical xv/xk/mask transfers have claimed their queue slots
            # (xq-rest is 3MB and isn't needed until qt=1, ~+70us)
            junk = singles.tile([128, 512], BF16, tag="junk", name="junk")
            for i in range(20):
                nc.gpsimd.memset(junk[:, :], 0.0)
            for c in range(CT):
                nc.gpsimd.dma_start(out=xq_sb[c][:, 512:2048],
                                    in_=xq_d[c][:, 512:2048])

            def bias_bc(col, n):
                bb = bqk_sb[:, col:col + 1]
                return bass.AP(tensor=bb.tensor, offset=bb.offset,
                               ap=[list(bb.ap[0]), [0, n]])

            def strided2(ap2d, stride):
                # [128, 64] AP -> [128, 2, 64] with the middle dim strided
                return bass.AP(tensor=ap2d.tensor, offset=ap2d.offset,
                               ap=[list(ap2d.ap[0]), [stride, 2],
                                   list(ap2d.ap[1])])

            # ---- single shared 8-bank PSUM pool for every phase ----
            with tc.tile_pool(name="scp", bufs=2, space="PSUM") as scp, \
                 tc.tile_pool(name="pvp", bufs=2, space="PSUM") as pvp:

                def psum_1024(i, name):
                    pool = scp if i % 2 == 0 else pvp
                    tag = "sc" if i % 2 == 0 else "po"
                    return pool.tile([128, 1024], F32, tag=tag, name=name)

                # K-projection first: 4 tiles = 8 single-bank (m,n)
                # accumulators, c-outer so matmuls chase the xk arrivals.
                kps = [psum_1024(i, f"kps{i}") for i in range(4)]

                def k_half(m, n):
                    t = kps[(m * QT + n) // 2]
                    off = ((m * QT + n) % 2) * 512
                    return t[:, off:off + 512]

                for i in range(10):  # HAM warmup while first DMAs land
                    m, n = (i % 8) // 4, i % 4
                    nc.tensor.matmul(k_half(m, n), warm[:, 0:128], warm[:, :],
                                     start=True, stop=True)
                for c in range(CT):
                    for m in range(2):
                        for n in range(QT):
                            nc.tensor.matmul(
                                k_half(m, n), wk(c, m),
                                xk_sb[c][:, n * 512:(n + 1) * 512],
                                start=(c == 0), stop=(c == CT - 1))
                # bias TTs n-outer so khT[*][0] (what scores kt0 needs) is
                # produced first
                for n in range(QT):
                    for m in range(2):
                        nc.vector.tensor_tensor(
                            out=khT[m][n], in0=k_half(m, n),
                            in1=bias_bc(2 + m, 512),
                            op=mybir.AluOpType.add)

                # V-projection runs as 4 waves of 4 kt on the po tag only,
                # interleaved into qt0's first attention iterations (below),
                # so the scores->exp stream never sits behind it in the PE
                # FIFO and the sc rotation stays free for scores.
                def v_wave(wave):
                    k0 = wave * 4
                    vps = [pvp.tile([128, 1024], F32, tag="po", name="vps")
                           for _ in range(2)]
                    for c in range(CT):
                        for i in range(4):
                            kt = k0 + i
                            nc.tensor.matmul(
                                vps[i // 2][:, (i % 2) * 512:(i % 2) * 512 + 256],
                                xv_sb[c][:, kt * 128:(kt + 1) * 128],
                                wv(c),
                                start=(c == 0), stop=(c == CT - 1))
                    for i in range(4):
                        kt = k0 + i
                        boff = (i % 2) * 512
                        # A-heads (psum cols 0:128) -> vha cols {0:64, 193:257}
                        # B-heads (128:256) -> {129:193, 322:386}
                        for (s0, d0) in ((0, 0), (128, 129)):
                            nc.vector.tensor_copy(
                                out=strided2(vha[kt][:, d0:d0 + 64], 193),
                                in_=strided2(
                                    vps[i // 2][:, boff + s0:boff + s0 + 64], 64))

                # Q-projection (n=0 up front; n+1 inside qt's kt-loop)
                def q_proj(n, m_list):
                    qps = scp.tile([128, 1024], F32, tag="sc", name="qps")
                    for m in m_list:
                        for c in range(CT):
                            nc.tensor.matmul(
                                qps[:, m * 512:(m + 1) * 512],
                                wq(c, m),
                                xq_sb[c][:, n * 512:(n + 1) * 512],
                                start=(c == 0), stop=(c == CT - 1))
                        nc.vector.tensor_tensor(
                            out=qhT[m][n], in0=qps[:, m * 512:(m + 1) * 512],
                            in1=bias_bc(m, 512),
                            op=mybir.AluOpType.add)

                # split m0/m1 into two allocations: scores(kt0, p0) only
                # needs qhT[m0][0], so its chain off the K TTs is shorter
                q_proj(0, [0])
                q_proj(0, [1])

                for qt in range(QT):
                    # allocated lazily: for qt0 the V-proj waves' po-tag
                    # allocations must come first in the rotation
                    po = []

                    def emit_pv(ktv, p, pt, po=po):
                        if not po:
                            po.extend(pvp.tile([128, 1024], F32, tag="po",
                                               name="po") for _ in range(2))
                        base = p * 193
                        nc.tensor.matmul(
                            po[p][0:65, 0:512],
                            vha[ktv][:, base:base + 65],
                            pt[:, 0:512],
                            start=(ktv == 0), stop=(ktv == KT - 1))
                        nc.tensor.matmul(
                            po[p][:, 512:1024],
                            vha[ktv][:, base + 65:base + 193],
                            pt[:, 512:1024],
                            start=(ktv == 0), stop=(ktv == KT - 1))

                    # PV is emitted `lag` kts late so the exp <- scores chain
                    # never sits behind PV/mask/V-proj work in the PE FIFO.
                    # qt0: lag 4, with the four V-proj waves interleaved after
                    # blocks kt0..kt3 (all waves emitted before the first PV
                    # -- its po-slot wait depends on wave 4's copies).
                    lag = 4 if qt == 0 else 1
                    pending = []
                    for kt in range(KT):
                        g, kl = kt // KG, kt % KG
                        if kl == 0:
                            # request masks 2 groups ahead (~18us of slack
                            # against the ~6us split-transfer time)
                            want_mask(qt * KG + g + 2)
                        mt = mask_tiles[(qt, g)]
                        m_ap = mt[:, kl * 512:(kl + 1) * 512]
                        mbc = bass.AP(
                            tensor=m_ap.tensor,
                            offset=m_ap.offset,
                            ap=[list(m_ap.ap[0]), [0, 2], list(m_ap.ap[1])])
                        pss, pts = [], []
                        for p in range(2):
                            ps = scp.tile([128, 1024], F32, tag="sc", name="ps")
                            for ab in range(2):
                                nc.tensor.matmul(
                                    ps[:, ab * 512:(ab + 1) * 512],
                                    khT[p][kt // 4][ab * 64:(ab + 1) * 64,
                                                    (kt % 4) * 128:(kt % 4 + 1) * 128],
                                    qhT[p][qt][ab * 64:(ab + 1) * 64, :],
                                    start=True, stop=True)
                            pss.append(ps)
                        for p in range(2):
                            pt = singles.tile([128, 1024], BF16, tag="pt",
                                              name="pt", bufs=12)
                            nc.scalar.activation(
                                out=pt, in_=pss[p],
                                func=mybir.ActivationFunctionType.Exp,
                                scale=float(SCALE))
                            nc.vector.tensor_tensor(
                                out=pt, in0=pt, in1=mbc,
                                op=mybir.AluOpType.mult)
                            pts.append(pt)
                        pending.append((kt, 0, pts[0]))
                        pending.append((kt, 1, pts[1]))
                        if qt == 0 and kt < 4:
                            v_wave(kt)
                        # drain at half-kt granularity: only 2 PV matmuls sit
                        # between consecutive scores blocks in the PE FIFO
                        while len(pending) > 2 * lag:
                            ktv, pv_p, pv_pt = pending.pop(0)
                            emit_pv(ktv, pv_p, pv_pt)
                        # all later Q-projections during qt0's ramp, where
                        # the exp stream is gated by the V-proj waves anyway
                        # (sc-slot holds are free there)
                        if qt == 0 and kt in (5, 7, 9):
                            q_proj((kt - 3) // 2, [0, 1])
                    for ktv, pv_p, pv_pt in pending:
                        emit_pv(ktv, pv_p, pv_pt)

                    # ---- per-qt: 1/sums chain first (it's the long pole),
                    # then evacuate po, then normalize ----
                    qsl = slice(qt * 512, (qt + 1) * 512)
                    ssl = slice(qt * 1024, (qt + 1) * 1024)
                    for p in range(2):
                        nc.vector.tensor_copy(
                            out=sums_stage[64:65, qt * 1024 + p * 512:
                                           qt * 1024 + (p + 1) * 512],
                            in_=po[p][64:65, 0:512])
                        nc.vector.tensor_copy(
                            out=sums_stage[32:33, qt * 1024 + p * 512:
                                           qt * 1024 + (p + 1) * 512],
                            in_=po[p][32:33, 512:1024])
                    # gather sums q-major into partitions: cols 0:8 = A(p0|p1),
                    # cols 8:16 = B(p0|p1); q = (part%64)*8 + col%8
                    recin = singles.tile([128, 16], BF16, tag="recin",
                                         name="recin", bufs=2)
                    nc.sync.dma_start(out=recin[:, 0:8],
                                      in_=sums_stage[64:65, ssl])
                    nc.sync.dma_start(out=recin[:, 8:16],
                                      in_=sums_stage[32:33, ssl])
                    for p in range(2):
                        nc.vector.tensor_copy(
                            out=OT[p][0:64, qsl], in_=po[p][0:64, 0:512])
                        nc.vector.tensor_copy(
                            out=OT[p][64:128, qsl], in_=po[p][64:128, 512:1024])
                    recout = singles.tile([128, 16], F32, tag="recout",
                                          name="recout", bufs=2)
                    nc.vector.reciprocal(out=recout, in_=recin)
                    recout_bf = singles.tile([128, 16], BF16, tag="recout_bf",
                                             name="recout_bf", bufs=2)
                    nc.vector.tensor_copy(out=recout_bf, in_=recout)
                    # scatter: rows 0:2 = A(p0,p1), rows 2:4 = B(p0,p1)
                    nc.sync.dma_start(out=rscr_d[qt, 0:2],
                                      in_=recout_bf[:, 0:8])
                    nc.sync.dma_start(out=rscr_d[qt, 2:4],
                                      in_=recout_bf[:, 8:16])
                    for p in range(2):
                        rbc = singles.tile([128, 512], BF16, tag="rbc",
                                           name="rbc", bufs=2)
                        # one DMA: partitions 0:64 <- A row p, 64:128 <- B row 2+p
                        srow = rscr_d[qt, p]
                        src_bc = bass.AP(
                            tensor=srow.tensor, offset=srow.offset,
                            ap=[[2 * 512, 2], [0, 64], list(srow.ap[-1])])
                        nc.sync.dma_start(out=rbc, in_=src_bc)
                        nc.vector.tensor_tensor(
                            out=OT[p][:, qsl], in0=OT[p][:, qsl], in1=rbc,
                            op=mybir.AluOpType.mult)

                # ---- output projection, q-quarter-major: n2<=2 only needs
                # qt0-2's normalization, so just 1/4 of it waits on the
                # serial qt3 1/sums chain ----
                idx = 0
                for n2 in range(QT):
                    for otp in range(4):
                        ps = psum_1024(idx, "psy")
                        for ho in range(2):
                            ot = otp * 2 + ho
                            for p in range(2):
                                nc.tensor.matmul(
                                    ps[:, ho * 512:(ho + 1) * 512],
                                    wo(p, ot),
                                    OT[p][:, n2 * 512:(n2 + 1) * 512],
                                    start=(p == 0), stop=(p == 1))
                        yt = singles.tile([128, 1024], BF16, tag="yt",
                                          name="yt", bufs=4)
                        # casts split across DVE and ACT, stores across
                        # sync and scalar
                        if idx % 2 == 0:
                            nc.vector.tensor_copy(out=yt, in_=ps)
                        else:
                            nc.scalar.copy(out=yt, in_=ps)
                        nc.sync.dma_start(out=yt_d[otp * 2, n2], in_=yt[:, 0:512])
                        nc.scalar.dma_start(out=yt_d[otp * 2 + 1, n2],
                                            in_=yt[:, 512:1024])
                        idx += 1
    nc.compile()
    return nc


_NC_CACHE = None


def get_nc():
    global _NC_CACHE
    if _NC_CACHE is None:
        _NC_CACHE = build_nc()
    return _NC_CACHE


def prep_in_maps(q, k, v, mask, Wq, bq, Wk, bk, Wv, bv, Wo, bo):
    q = np.asarray(q, np.float32)
    k = np.asarray(k, np.float32)
    v = np.asarray(v, np.float32)
    mask = np.asarray(mask)
    WqT = np.asarray(Wq, np.float32).T
    WkT = np.asarray(Wk, np.float32).T
    WvT = np.asarray(Wv, np.float32).T
    WoT = np.asarray(Wo, np.float32).T
    bq = np.asarray(bq, np.float32)
    bk = np.asarray(bk, np.float32)

    xT = {}
    keepT = {}
    for b in range(B):
        xT[b] = tuple(
            np.ascontiguousarray(arr[b].T).astype(NP_BF16).reshape(CT, 128, 2048)
            for arr in (q, k, v))
        mt = np.ascontiguousarray((~mask[b, 0]).T.astype(np.float32)).astype(NP_BF16)
        # [kv, q] -> [KT, QT, 128, 512] -> [QT, KG, 128, KG*512] kt-interleaved
        t = mt.reshape(KT, 128, QT, 512).transpose(0, 2, 1, 3)
        keepT[b] = np.ascontiguousarray(
            t.reshape(KT // KG, KG, QT, 128, 512)
            .transpose(2, 0, 3, 1, 4).reshape(QT, KG, 128, KG * 512))

    in_maps = []
    for c in range(N_CORES):
        b = c // 4
        ho = c % 4
        dsl = slice(ho * 256, ho * 256 + 256)
        xq, xk, xv = xT[b]
        wq_t = np.asarray(WqT[:, dsl], np.float32).astype(NP_BF16).reshape(CT, 128, 256)
        wk_t = np.asarray(WkT[:, dsl], np.float32).astype(NP_BF16).reshape(CT, 128, 256)
        # wv columns permuted to (p0A, p1A, p0B, p1B) head order
        wv_c = np.asarray(WvT[:, dsl], np.float32).astype(NP_BF16).reshape(
            CT, 128, 4, 64)[:, :, [0, 2, 1, 3], :].reshape(CT, 128, 256)
        # wo_flat: [128, 2048] = [wo_m0 | wo_m1], sliced 256 per chunk
        wo_t = np.ascontiguousarray(WoT[dsl, :]).astype(NP_BF16)
        wo_flat = wo_t.reshape(2, 128, 1024).transpose(1, 0, 2).reshape(128, 2048)
        wo_c = wo_flat.reshape(128, CT, 256).transpose(1, 0, 2)
        xq_aug = np.concatenate([xq, wq_t], axis=2)
        xk_aug = np.concatenate([xk, wk_t], axis=2)
        xv_aug = np.concatenate([xv, wv_c, wo_c], axis=2)
        bqk = np.stack([bq[dsl][:128], bq[dsl][128:],
                        bk[dsl][:128], bk[dsl][128:]], axis=1)
        in_maps.append({
            "xq": np.ascontiguousarray(xq_aug),
            "xk": np.ascontiguousarray(xk_aug),
            "xv": np.ascontiguousarray(xv_aug),
            "bqk": np.ascontiguousarray(bqk).astype(np.float32),
            "mk": keepT[b],
        })
    return in_maps


def gather_output(results, bo, bv, Wo):
    bo = np.asarray(bo, np.float64)
    bv = np.asarray(bv, np.float64)
    Wo = np.asarray(Wo, np.float64)
    y = np.zeros((B, S, DIM), np.float32)
    for c in range(N_CORES):
        yt = np.asarray(results[c]["yt"], np.float32)  # [8, 4, 128, 512]
        yT = yt.transpose(0, 2, 1, 3).reshape(DIM, S)
        y[c // 4] += yT.T
    # V bias folded out of the kernel: O_norm = PV/sums + bv, so the bv term
    # contributes the constant row bv @ Wo.T to every output position.
    y += (bo + bv @ Wo.T).astype(np.float32)[None, None, :]
    return y


def kernel(**inputs):
    nc = get_nc()
    in_maps = prep_in_maps(**{k_: inputs[k_] for k_ in (
        "q", "k", "v", "mask", "Wq", "bq", "Wk", "bk", "Wv", "bv", "Wo", "bo")})
    res = bass_utils.run_bass_kernel_spmd(nc, in_maps, list(range(N_CORES)))
    return gather_output(res.results, inputs["bo"], inputs["bv"], inputs["Wo"])


# revision 39
# speedup vs baseline: 1.1430x; 1.1430x over previous
"""MultiHeadAttention Trainium2 Bass kernel (8-core SPMD), v4.

Problem: B=2, S=2048, DIM=1024, H=16 heads (dh=64), fp32 reference.
Sharding: core c handles batch b = c//4 and 4 heads ho = 4*(c%4)..+4
(data-parallel over batch x tensor-parallel over heads).

Key HW facts driving the structure (measured on this part):
  - Each dma_start costs ~650ns of serialized trigger time on its issuing
    engine (only sync/scalar/gpsimd can trigger); per-queue DMA bandwidth
    caps at ~23GB/s regardless of packet size; 16 HW queues ~368GB/s
    aggregate. So: weights are PACKED INTO the x tensors host-side (zero
    extra triggers), each x tensor is split into 16 transfer pieces to
    fill all queues, and triggers are spread across sync+scalar+gpsimd.
  - ACT exp is the compute wall: 128 x (352+1024)/1.2 = 147us. Everything
    else (PE ~130us, DVE ~115us, 25MB DMA) must hide under it.
  - One PSUM accumulation group per 2KB bank; 8 banks total. All phases
    share one pool pair (tags sc/po, 4 tiles of [128,1024]). K-projection
    runs c-outer over all 8 banks chasing the xk arrivals; V-projection
    runs as 4 waves of 4 kt on the po tag only, interleaved into qt0's
    first attention iterations, so the scores->exp stream never sits
    behind it in the PE FIFO and the sc rotation stays free for scores.
  - Load order: xk/xv interleaved 2:1 on sync (xk = exp critical path
    finishes first, xv right behind for the V waves), qt0 masks split in
    4 pieces each, xq-n0+wq via gpsimd, xq-rest delayed behind a gpsimd
    stall. exp starts ~51us in; masks prefetch 2 groups ahead in-loop.
  - PV is emitted `lag` kts late (4 for qt0, 1 after) so exp<-scores
    never waits on PV/mask work through the PE FIFO.
  - V bias is folded out: O_norm = PV_raw/sums + bv, so bv only shifts
    the output by bv @ Wo.T, added host-side. Saves 64 DVE bias ops.
  - Per-qt softmax normalization (reciprocal via DMA gather/broadcast)
    overlaps the next qt's attention; all Q-projections for qt>=1 are
    emitted during qt0's ramp where exp is gated by the V waves anyway.
  - Output projection is q-quarter-major so only 1/4 of it waits on the
    serial qt3 1/sums chain; casts split across DVE+ACT, stores across
    sync+scalar queues.

Measured: 268us (baseline 332us/300us), rel err 2.4e-3 vs fp32 ref.
"""

import os
import sys

sys.path.insert(0, "/opt/trn_rl_repo")
os.environ.setdefault("MYCRO_LOCAL_CACHE", "1")

import numpy as np

import concourse.bass as bass
import concourse.bacc as bacc
import concourse.tile as tile
from concourse import mybir
from concourse import bass_utils

F32 = mybir.dt.float32
BF16 = mybir.dt.bfloat16
NP_BF16 = mybir.dt.np(BF16)

B, S, DIM = 2, 2048, 1024
H = 16
DH = 64
SCALE = 1.0 / (DIM ** 0.5)
N_CORES = 8
HPC = 4          # heads per core
QT = S // 512    # 4 q-chunks of 512
KT = S // 128    # 16 k-tiles of 128
CT = DIM // 128  # 8 contraction tiles for projections
KG = 4           # kt-tiles per mask group

XKW = 2048 + 256          # xk chunk | wk chunk
XVW = 2048 + 256 + 256    # xv chunk | wv chunk | wo_flat chunk
XQW = 2048 + 256          # xq chunk | wq chunk

# vh_aug per-kt layout: per pair p (2 local pairs):
#   A block: [vh_A(64) | ones(1)]                 at cols p*193 + [0, 65)
#   B block: [zeros(32) | ones(1) | zeros(31) | vh_B(64)] at cols p*193 + [65, 193)
# wv host columns are permuted (p0A, p1A, p0B, p1B) so the psum->vha
# evacuation is 2 strided copies instead of 4.
VHA_W = 386


def build_nc():
    nc = bacc.Bacc("TRN2", target_bir_lowering=False)

    xq_d = nc.declare_dram_parameter("xq", [CT, 128, XQW], BF16, isOutput=False)
    xk_d = nc.declare_dram_parameter("xk", [CT, 128, XKW], BF16, isOutput=False)
    xv_d = nc.declare_dram_parameter("xv", [CT, 128, XVW], BF16, isOutput=False)
    bqk_d = nc.declare_dram_parameter("bqk", [128, 4], F32, isOutput=False)
    mk_d = nc.declare_dram_parameter("mk", [QT, KG, 128, 2048], BF16, isOutput=False)
    yt_d = nc.declare_dram_parameter("yt", [8, 4, 128, 512], BF16, isOutput=True)
    # per-qt reciprocal staging: rows 0-1 = A-heads (p0,p1), 2-3 = B-heads
    rscr_d = nc.dram_tensor("rscr", [QT, 4, 512], BF16)

    with tile.TileContext(nc) as tc:
        with tc.tile_pool(name="persist", bufs=1) as singles:
            # PE warmup filler source -- first gpsimd op so it's ready fast
            warm = singles.tile([128, 512], BF16, tag="warm", name="warm")
            nc.gpsimd.memset(warm[:, :], 0.0)

            # preload the exp table set on ACT before its DMA triggers
            etw = singles.tile([1, 2], BF16, tag="etw", name="etw")
            nc.scalar.activation(out=etw, in_=warm[0:1, 0:2],
                                 func=mybir.ActivationFunctionType.Exp)

            def alloc_x(tag, width):
                return [singles.tile([128, width], BF16, tag=f"{tag}{c}",
                                     name=f"{tag}{c}") for c in range(CT)]

            # xv on sync, xk on scalar: two parallel trigger streams from t=0.
            # Queue collisions between the streams average out (all of this
            # data is needed before attention starts anyway).
            # xk first (the scores->exp critical path), then qt0's first two
            # masks, then xv (the V-proj path has pt-buffer slack to absorb
            # its later arrival). All on sync's queue round-robin so the
            # per-queue FIFO order matches this priority.
            bqk_sb = singles.tile([128, 4], F32, tag="bqk", name="bqk")
            nc.sync.dma_start(out=bqk_sb, in_=bqk_d[:, :])
            # xk and xv interleaved 2:1 on sync's queue round-robin: xk
            # (scores critical path) finishes ~2/3 through the stream, and
            # xv is close enough behind that the V-proj waves (which need
            # full xv) land just as the attention pipeline first needs vha
            xk_sb = alloc_x("xk", XKW)
            xv_sb = alloc_x("xv", XVW)
            xv_pieces = [(c, 0, 1280) for c in range(CT)] + \
                        [(c, 1280, XVW) for c in range(CT)]
            for c in range(CT):
                nc.sync.dma_start(out=xk_sb[c][:, 0:1152], in_=xk_d[c][:, 0:1152])
                nc.sync.dma_start(out=xk_sb[c][:, 1152:XKW],
                                  in_=xk_d[c][:, 1152:XKW])
                vc, v0, v1 = xv_pieces[c]
                nc.sync.dma_start(out=xv_sb[vc][:, v0:v1], in_=xv_d[vc][:, v0:v1])
            for vc, v0, v1 in xv_pieces[CT:]:
                nc.sync.dma_start(out=xv_sb[vc][:, v0:v1], in_=xv_d[vc][:, v0:v1])

            mask_tiles = {}

            def load_mask(qt, g):
                # 4-piece split: a whole 512KB tile on one ~23GB/s queue
                # takes 22us and always arrives late
                t = singles.tile([128, 2048], BF16, tag="mask", name="mask",
                                 bufs=3)
                for i in range(4):
                    nc.sync.dma_start(
                        out=t[:, i * 512:(i + 1) * 512],
                        in_=mk_d[qt, g][:, i * 512:(i + 1) * 512])
                mask_tiles[(qt, g)] = t

            def want_mask(gi):
                if gi < QT * KG:
                    qt, g = gi // KG, gi % KG
                    if (qt, g) not in mask_tiles:
                        load_mask(qt, g)

            want_mask(0)
            want_mask(1)

            # xq n=0 cols + wq via gpsimd triggers (interleaves with xk/xv)
            xq_sb = alloc_x("xq", XQW)
            for c in range(CT):
                nc.gpsimd.dma_start(out=xq_sb[c][:, 0:512], in_=xq_d[c][:, 0:512])
            for c in range(CT):
                nc.gpsimd.dma_start(out=xq_sb[c][:, 2048:XQW],
                                    in_=xq_d[c][:, 2048:XQW])

            def wk(c, m):
                return xk_sb[c][:, 2048 + m * 128:2048 + (m + 1) * 128]

            def wv(c):
                return xv_sb[c][:, 2048:2304]

            def wo(p, ot):
                col = p * 1024 + ot * 128
                return xv_sb[col // 256][:, 2304 + col % 256:2304 + col % 256 + 128]

            def wq(c, m):
                return xq_sb[c][:, 2048 + m * 128:2048 + (m + 1) * 128]

            # ---- persistent intermediates ----
            qhT = [[singles.tile([128, 512], BF16, tag=f"qhT{m}_{n}",
                                 name=f"qhT{m}_{n}") for n in range(QT)]
                   for m in range(2)]
            khT = [[singles.tile([128, 512], BF16, tag=f"khT{m}_{n}",
                                 name=f"khT{m}_{n}") for n in range(QT)]
                   for m in range(2)]
            OT = [singles.tile([128, S], BF16, tag=f"OT{m}", name=f"OT{m}") for m in range(2)]
            vha = [singles.tile([128, VHA_W], BF16, tag=f"vha{kt}",
                                name=f"vha{kt}") for kt in range(KT)]
            # cols: qt*1024 + p*512 + q  (p-adjacent: flat per-qt gather reads)
            sums_stage = singles.tile([128, 2 * S], BF16, tag="sums_stage",
                                      name="sums_stage")

            for kt in range(KT):
                for p in range(2):
                    base = p * 193
                    nc.gpsimd.memset(vha[kt][:, base + 64:base + 65], 1.0)
                    nc.gpsimd.memset(vha[kt][:, base + 97:base + 98], 1.0)
                    nc.gpsimd.memset(vha[kt][:, base + 65:base + 97], 0.0)
                    nc.gpsimd.memset(vha[kt][:, base + 98:base + 129], 0.0)

            # stall gpsimd ~12us so the xq n=1..3 triggers fire after the
            # critical xv/xk/mask transfers have claimed their queue slots
            # (xq-rest is 3MB and isn't needed until qt=1, ~+70us)
            junk = singles.tile([128, 512], BF16, tag="junk", name="junk")
            for i in range(20):
                nc.gpsimd.memset(junk[:, :], 0.0)
            for c in range(CT):
                nc.gpsimd.dma_start(out=xq_sb[c][:, 512:2048],
                                    in_=xq_d[c][:, 512:2048])

            def bias_bc(col, n):
                bb = bqk_sb[:, col:col + 1]
                return bass.AP(tensor=bb.tensor, offset=bb.offset,
                               ap=[list(bb.ap[0]), [0, n]])

            def strided2(ap2d, stride):
                # [128, 64] AP -> [128, 2, 64] with the middle dim strided
                return bass.AP(tensor=ap2d.tensor, offset=ap2d.offset,
                               ap=[list(ap2d.ap[0]), [stride, 2],
                                   list(ap2d.ap[1])])

            # ---- single shared 8-bank PSUM pool for every phase ----
            with tc.tile_pool(name="scp", bufs=2, space="PSUM") as scp, \
                 tc.tile_pool(name="pvp", bufs=2, space="PSUM") as pvp:

                def psum_1024(i, name):
                    pool = scp if i % 2 == 0 else pvp
                    tag = "sc" if i % 2 == 0 else "po"
                    return pool.tile([128, 1024], F32, tag=tag, name=name)

                # K-projection first: 4 tiles = 8 single-bank (m,n)
                # accumulators, c-outer so matmuls chase the xk arrivals.
                kps = [psum_1024(i, f"kps{i}") for i in range(4)]

                def k_half(m, n):
                    t = kps[(m * QT + n) // 2]
                    off = ((m * QT + n) % 2) * 512
                    return t[:, off:off + 512]

                for i in range(10):  # HAM warmup while first DMAs land
                    m, n = (i % 8) // 4, i % 4
                    nc.tensor.matmul(k_half(m, n), warm[:, 0:128], warm[:, :],
                                     start=True, stop=True)
                for c in range(CT):
                    for m in range(2):
                        for n in range(QT):
                            nc.tensor.matmul(
                                k_half(m, n), wk(c, m),
                                xk_sb[c][:, n * 512:(n + 1) * 512],
                                start=(c == 0), stop=(c == CT - 1))
                # bias TTs n-outer so khT[*][0] (what scores kt0 needs) is
                # produced first
                for n in range(QT):
                    for m in range(2):
                        nc.vector.tensor_tensor(
                            out=khT[m][n], in0=k_half(m, n),
                            in1=bias_bc(2 + m, 512),
                            op=mybir.AluOpType.add)

                # V-projection runs as 4 waves of 4 kt on the po tag only,
                # interleaved into qt0's first attention iterations (below),
                # so the scores->exp stream never sits behind it in the PE
                # FIFO and the sc rotation stays free for scores.
                def v_wave(wave):
                    k0 = wave * 4
                    vps = [pvp.tile([128, 1024], F32, tag="po", name="vps")
                           for _ in range(2)]
                    for c in range(CT):
                        for i in range(4):
                            kt = k0 + i
                            nc.tensor.matmul(
                                vps[i // 2][:, (i % 2) * 512:(i % 2) * 512 + 256],
                                xv_sb[c][:, kt * 128:(kt + 1) * 128],
                                wv(c),
                                start=(c == 0), stop=(c == CT - 1))
                    for i in range(4):
                        kt = k0 + i
                        boff = (i % 2) * 512
                        # A-heads (psum cols 0:128) -> vha cols {0:64, 193:257}
                        # B-heads (128:256) -> {129:193, 322:386}
                        for (s0, d0) in ((0, 0), (128, 129)):
                            nc.vector.tensor_copy(
                                out=strided2(vha[kt][:, d0:d0 + 64], 193),
                                in_=strided2(
                                    vps[i // 2][:, boff + s0:boff + s0 + 64], 64))

                # Q-projection (n=0 up front; n+1 inside qt's kt-loop)
                def q_proj(n, m_list):
                    qps = scp.tile([128, 1024], F32, tag="sc", name="qps")
                    for m in m_list:
                        for c in range(CT):
                            nc.tensor.matmul(
                                qps[:, m * 512:(m + 1) * 512],
                                wq(c, m),
                                xq_sb[c][:, n * 512:(n + 1) * 512],
                                start=(c == 0), stop=(c == CT - 1))
                        nc.vector.tensor_tensor(
                            out=qhT[m][n], in0=qps[:, m * 512:(m + 1) * 512],
                            in1=bias_bc(m, 512),
                            op=mybir.AluOpType.add)

                # split m0/m1 into two allocations: scores(kt0, p0) only
                # needs qhT[m0][0], so its chain off the K TTs is shorter
                q_proj(0, [0])
                q_proj(0, [1])

                for qt in range(QT):
                    # allocated lazily: for qt0 the V-proj waves' po-tag
                    # allocations must come first in the rotation
                    po = []

                    def emit_pv(ktv, p, pt, po=po):
                        if not po:
                            po.extend(pvp.tile([128, 1024], F32, tag="po",
                                               name="po") for _ in range(2))
                        base = p * 193
                        nc.tensor.matmul(
                            po[p][0:65, 0:512],
                            vha[ktv][:, base:base + 65],
                            pt[:, 0:512],
                            start=(ktv == 0), stop=(ktv == KT - 1))
                        nc.tensor.matmul(
                            po[p][:, 512:1024],
                            vha[ktv][:, base + 65:base + 193],
                            pt[:, 512:1024],
                            start=(ktv == 0), stop=(ktv == KT - 1))

                    # PV is emitted `lag` kts late so the exp <- scores chain
                    # never sits behind PV/mask/V-proj work in the PE FIFO.
                    # qt0: lag 4, with the four V-proj waves interleaved after
                    # blocks kt0..kt3 (all waves emitted before the first PV
                    # -- its po-slot wait depends on wave 4's copies).
                    lag = 4 if qt == 0 else 1
                    pending = []
                    for kt in range(KT):
                        g, kl = kt // KG, kt % KG
                        if kl == 0:
                            # request masks 2 groups ahead (~18us of slack
                            # against the ~6us split-transfer time)
                            want_mask(qt * KG + g + 2)
                        mt = mask_tiles[(qt, g)]
                        m_ap = mt[:, kl * 512:(kl + 1) * 512]
                        mbc = bass.AP(
                            tensor=m_ap.tensor,
                            offset=m_ap.offset,
                            ap=[list(m_ap.ap[0]), [0, 2], list(m_ap.ap[1])])
                        pss, pts = [], []
                        for p in range(2):
                            ps = scp.tile([128, 1024], F32, tag="sc", name="ps")
                            for ab in range(2):
                                nc.tensor.matmul(
                                    ps[:, ab * 512:(ab + 1) * 512],
                                    khT[p][kt // 4][ab * 64:(ab + 1) * 64,
                                                    (kt % 4) * 128:(kt % 4 + 1) * 128],
                                    qhT[p][qt][ab * 64:(ab + 1) * 64, :],
                                    start=True, stop=True)
                            pss.append(ps)
                        for p in range(2):
                            pt = singles.tile([128, 1024], BF16, tag="pt",
                                              name="pt", bufs=10)
                            nc.scalar.activation(
                                out=pt, in_=pss[p],
                                func=mybir.ActivationFunctionType.Exp,
                                scale=float(SCALE))
                            nc.vector.tensor_tensor(
                                out=pt, in0=pt, in1=mbc,
                                op=mybir.AluOpType.mult)
                            pts.append(pt)
                        pending.append((kt, 0, pts[0]))
                        pending.append((kt, 1, pts[1]))
                        if qt == 0 and kt < 4:
                            v_wave(kt)
                        # drain at half-kt granularity: only 2 PV matmuls sit
                        # between consecutive scores blocks in the PE FIFO
                        while len(pending) > 2 * lag:
                            ktv, pv_p, pv_pt = pending.pop(0)
                            emit_pv(ktv, pv_p, pv_pt)
                        # all later Q-projections during qt0's ramp, where
                        # the exp stream is gated by the V-proj waves anyway
                        # (sc-slot holds are free there)
                        if qt == 0 and kt in (5, 7, 9):
                            q_proj((kt - 3) // 2, [0, 1])
                    for ktv, pv_p, pv_pt in pending:
                        emit_pv(ktv, pv_p, pv_pt)

                    # ---- per-qt: 1/sums chain first (it's the long pole),
                    # then evacuate po, then normalize ----
                    qsl = slice(qt * 512, (qt + 1) * 512)
                    ssl = slice(qt * 1024, (qt + 1) * 1024)
                    for p in range(2):
                        nc.vector.tensor_copy(
                            out=sums_stage[64:65, qt * 1024 + p * 512:
                                           qt * 1024 + (p + 1) * 512],
                            in_=po[p][64:65, 0:512])
                        nc.vector.tensor_copy(
                            out=sums_stage[32:33, qt * 1024 + p * 512:
                                           qt * 1024 + (p + 1) * 512],
                            in_=po[p][32:33, 512:1024])
                    # gather sums q-major into partitions: cols 0:8 = A(p0|p1),
                    # cols 8:16 = B(p0|p1); q = (part%64)*8 + col%8
                    recin = singles.tile([128, 16], BF16, tag="recin",
                                         name="recin", bufs=2)
                    nc.sync.dma_start(out=recin[:, 0:8],
                                      in_=sums_stage[64:65, ssl])
                    nc.sync.dma_start(out=recin[:, 8:16],
                                      in_=sums_stage[32:33, ssl])
                    for p in range(2):
                        nc.vector.tensor_copy(
                            out=OT[p][0:64, qsl], in_=po[p][0:64, 0:512])
                        nc.vector.tensor_copy(
                            out=OT[p][64:128, qsl], in_=po[p][64:128, 512:1024])
                    recout = singles.tile([128, 16], F32, tag="recout",
                                          name="recout", bufs=2)
                    nc.vector.reciprocal(out=recout, in_=recin)
                    recout_bf = singles.tile([128, 16], BF16, tag="recout_bf",
                                             name="recout_bf", bufs=2)
                    nc.vector.tensor_copy(out=recout_bf, in_=recout)
                    # scatter: rows 0:2 = A(p0,p1), rows 2:4 = B(p0,p1)
                    nc.sync.dma_start(out=rscr_d[qt, 0:2],
                                      in_=recout_bf[:, 0:8])
                    nc.sync.dma_start(out=rscr_d[qt, 2:4],
                                      in_=recout_bf[:, 8:16])
                    for p in range(2):
                        rbc = singles.tile([128, 512], BF16, tag="rbc",
                                           name="rbc", bufs=4)
                        # one DMA: partitions 0:64 <- A row p, 64:128 <- B row 2+p
                        srow = rscr_d[qt, p]
                        src_bc = bass.AP(
                            tensor=srow.tensor, offset=srow.offset,
                            ap=[[2 * 512, 2], [0, 64], list(srow.ap[-1])])
                        nc.sync.dma_start(out=rbc, in_=src_bc)
                        nc.vector.tensor_tensor(
                            out=OT[p][:, qsl], in0=OT[p][:, qsl], in1=rbc,
                            op=mybir.AluOpType.mult)

                # ---- output projection, q-quarter-major: n2<=2 only needs
                # qt0-2's normalization, so just 1/4 of it waits on the
                # serial qt3 1/sums chain ----
                idx = 0
                for n2 in range(QT):
                    for otp in range(4):
                        ps = psum_1024(idx, "psy")
                        for ho in range(2):
                            ot = otp * 2 + ho
                            for p in range(2):
                                nc.tensor.matmul(
                                    ps[:, ho * 512:(ho + 1) * 512],
                                    wo(p, ot),
                                    OT[p][:, n2 * 512:(n2 + 1) * 512],
                                    start=(p == 0), stop=(p == 1))
                        yt = singles.tile([128, 1024], BF16, tag="yt",
                                          name="yt", bufs=4)
                        # casts split across DVE and ACT, stores across
                        # sync and scalar
                        if idx % 2 == 0:
                            nc.vector.tensor_copy(out=yt, in_=ps)
                        else:
                            nc.scalar.copy(out=yt, in_=ps)
                        nc.sync.dma_start(out=yt_d[otp * 2, n2], in_=yt[:, 0:512])
                        nc.scalar.dma_start(out=yt_d[otp * 2 + 1, n2],
                                            in_=yt[:, 512:1024])
                        idx += 1
    nc.compile()
    return nc


_NC_CACHE = None


def get_nc():
    global _NC_CACHE
    if _NC_CACHE is None:
        _NC_CACHE = build_nc()
    return _NC_CACHE


def prep_in_maps(q, k, v, mask, Wq, bq, Wk, bk, Wv, bv, Wo, bo):
    q = np.asarray(q, np.float32)
    k = np.asarray(k, np.float32)
    v = np.asarray(v, np.float32)
    mask = np.asarray(mask)
    WqT = np.asarray(Wq, np.float32).T
    WkT = np.asarray(Wk, np.float32).T
    WvT = np.asarray(Wv, np.float32).T
    WoT = np.asarray(Wo, np.float32).T
    bq = np.asarray(bq, np.float32)
    bk = np.asarray(bk, np.float32)

    xT = {}
    keepT = {}
    for b in range(B):
        xT[b] = tuple(
            np.ascontiguousarray(arr[b].T).astype(NP_BF16).reshape(CT, 128, 2048)
            for arr in (q, k, v))
        mt = np.ascontiguousarray((~mask[b, 0]).T.astype(np.float32)).astype(NP_BF16)
        # [kv, q] -> [KT, QT, 128, 512] -> [QT, KG, 128, KG*512] kt-interleaved
        t = mt.reshape(KT, 128, QT, 512).transpose(0, 2, 1, 3)
        keepT[b] = np.ascontiguousarray(
            t.reshape(KT // KG, KG, QT, 128, 512)
            .transpose(2, 0, 3, 1, 4).reshape(QT, KG, 128, KG * 512))

    in_maps = []
    for c in range(N_CORES):
        b = c // 4
        ho = c % 4
        dsl = slice(ho * 256, ho * 256 + 256)
        xq, xk, xv = xT[b]
        wq_t = np.asarray(WqT[:, dsl], np.float32).astype(NP_BF16).reshape(CT, 128, 256)
        wk_t = np.asarray(WkT[:, dsl], np.float32).astype(NP_BF16).reshape(CT, 128, 256)
        # wv columns permuted to (p0A, p1A, p0B, p1B) head order
        wv_c = np.asarray(WvT[:, dsl], np.float32).astype(NP_BF16).reshape(
            CT, 128, 4, 64)[:, :, [0, 2, 1, 3], :].reshape(CT, 128, 256)
        # wo_flat: [128, 2048] = [wo_m0 | wo_m1], sliced 256 per chunk
        wo_t = np.ascontiguousarray(WoT[dsl, :]).astype(NP_BF16)
        wo_flat = wo_t.reshape(2, 128, 1024).transpose(1, 0, 2).reshape(128, 2048)
        wo_c = wo_flat.reshape(128, CT, 256).transpose(1, 0, 2)
        xq_aug = np.concatenate([xq, wq_t], axis=2)
        xk_aug = np.concatenate([xk, wk_t], axis=2)
        xv_aug = np.concatenate([xv, wv_c, wo_c], axis=2)
        bqk = np.stack([bq[dsl][:128], bq[dsl][128:],
                        bk[dsl][:128], bk[dsl][128:]], axis=1)
        in_maps.append({
            "xq": np.ascontiguousarray(xq_aug),
            "xk": np.ascontiguousarray(xk_aug),
            "xv": np.ascontiguousarray(xv_aug),
            "bqk": np.ascontiguousarray(bqk).astype(np.float32),
            "mk": keepT[b],
        })
    return in_maps


def gather_output(results, bo, bv, Wo):
    bo = np.asarray(bo, np.float64)
    bv = np.asarray(bv, np.float64)
    Wo = np.asarray(Wo, np.float64)
    y = np.zeros((B, S, DIM), np.float32)
    for c in range(N_CORES):
        yt = np.asarray(results[c]["yt"], np.float32)  # [8, 4, 128, 512]
        yT = yt.transpose(0, 2, 1, 3).reshape(DIM, S)
        y[c // 4] += yT.T
    # V bias folded out of the kernel: O_norm = PV/sums + bv, so the bv term
    # contributes the constant row bv @ Wo.T to every output position.
    y += (bo + bv @ Wo.T).astype(np.float32)[None, None, :]
    return y


def kernel(**inputs):
    nc = get_nc()
    in_maps = prep_in_maps(**{k_: inputs[k_] for k_ in (
        "q", "k", "v", "mask", "Wq", "bq", "Wk", "bk", "Wv", "bv", "Wo", "bo")})
    res = bass_utils.run_bass_kernel_spmd(nc, in_maps, list(range(N_CORES)))
    return gather_output(res.results, inputs["bo"], inputs["bv"], inputs["Wo"])
